# revision 24
# baseline (speedup 1.0000x reference)
import sys
from contextlib import ExitStack

import numpy as np
import ml_dtypes

sys.path.insert(0, "/opt/trn_rl_repo")

try:
    import jax
    jax.config.update("jax_compilation_cache_dir", "/tmp/jax_cc_cache")
    jax.config.update("jax_persistent_cache_min_compile_time_secs", 0.0)
    jax.config.update("jax_persistent_cache_min_entry_size_bytes", 0)
except Exception:
    pass

import concourse.bass as bass
import concourse.tile as tile
from concourse import bacc, mybir
from concourse.bass_utils import run_bass_kernel_spmd

B, H, W, CH = 4, 80, 80, 256
NCLS, DIM = 22, 256
ROWS = 40            # rows per core
NPIX = ROWS * W      # 3200 output pixels per core
NT = (ROWS + 2) * W + 2   # 3362 strip positions (1 halo row each side + 1 elem pad)
NTILE = NPIX // 128  # 25 output tiles of 128 pixels
SELW = 9 * 128       # per-tile selp row width (k-major, pixel minor)
# merged-input column layout (int8 bytes): [x strip int8 | sel bf16 | w bf16]
XC = 2 * NT                   # 6724 int8 cols of per-pixel-quantized x
SC = NTILE * SELW * 2 // 128  # 450 byte-cols of sel (bf16)
WC = 16 * 18 * DIM * 2 // 128  # 1152 byte-cols of weight shard (bf16)
TOTC = XC + SC + WC           # 8326
QROWS = NPIX + 50         # int8 output rows + 50 rows holding f32 scales
F32 = mybir.dt.float32
I8 = mybir.dt.int8
BF16 = mybir.dt.bfloat16
BF16NP = ml_dtypes.bfloat16


def _build_nc():
    nc = bacc.Bacc("TRN2", target_bir_lowering=False, debug=False,
                   enable_asserts=True, num_devices=8)
    # single merged int8 input: quantized x cols [0:XC), sel bytes
    # [XC:XC+SC), 1/8 weight-shard bytes [XC+SC:TOTC) (AllGather assembles)
    inx_d = nc.dram_tensor("inx", [128, TOTC], I8, kind="ExternalInput").ap()
    # single merged output: rows [0:NPIX) per-pixel int8 quantized values,
    # rows [NPIX:QROWS) the f32 per-pixel quant steps (bitcast to bytes)
    qout_d = nc.dram_tensor("qout", [QROWS, DIM], I8,
                            kind="ExternalOutput").ap()

    with tile.TileContext(nc) as tc, ExitStack() as ctx:
        dramp = ctx.enter_context(tc.tile_pool(name="dramp", bufs=1,
                                               space="DRAM"))
        xp = ctx.enter_context(tc.tile_pool(name="xp", bufs=1))
        wp = ctx.enter_context(tc.tile_pool(name="wp", bufs=1))
        stp = ctx.enter_context(tc.tile_pool(name="stp", bufs=1))
        Sp = ctx.enter_context(tc.tile_pool(name="Sp", bufs=3))
        xtsp = ctx.enter_context(tc.tile_pool(name="xtsp", bufs=3))
        outp = ctx.enter_context(tc.tile_pool(name="outp", bufs=3))
        rmp = ctx.enter_context(tc.tile_pool(name="rmp", bufs=3))
        sckp = ctx.enter_context(tc.tile_pool(name="sckp", bufs=1))
        zp = ctx.enter_context(tc.tile_pool(name="zp", bufs=6, space="PSUM"))

        xt = xp.tile([128, XC], I8)
        wt = wp.tile([128, 18 * DIM], BF16)
        selt = stp.tile([1, NTILE * SELW], BF16)

        win = dramp.tile([16, 18 * DIM], BF16)
        wfull = dramp.tile([128, 18 * DIM], BF16)
        nc.gpsimd.dma_start(win[:], inx_d[:, XC + SC:TOTC].bitcast(BF16))
        nc.gpsimd.collective_compute(
            "AllGather", mybir.AluOpType.bypass,
            replica_groups=[list(range(8))],
            ins=[win.opt()], outs=[wfull.opt()])

        nc.sync.dma_start(selt[:], inx_d[:, XC:XC + SC].bitcast(BF16))
        # x chunk 0 first (tile 0's multiply needs it), then weights, then rest
        bnds = [0, 850, 1700, 2550, NT]
        for h in range(2):
            nc.sync.dma_start(xt[:, h * NT:h * NT + bnds[1]],
                              inx_d[:, h * NT:h * NT + bnds[1]])
        for k in range(9):
            nc.sync.dma_start(wt[:, k * 512:(k + 1) * 512],
                              wfull[:, k * 512:(k + 1) * 512])
        for ci in range(1, 4):
            for h in range(2):
                a, b = h * NT + bnds[ci], h * NT + bnds[ci + 1]
                nc.sync.dma_start(xt[:, a:b], inx_d[:, a:b])

        sck = sckp.tile([128, NTILE], F32)
        for j in range(NTILE):
            S = Sp.tile([128, SELW], BF16)
            nc.gpsimd.partition_broadcast(
                S[:], selt[0:1, j * SELW:(j + 1) * SELW])
            xts = xtsp.tile([128, 2 * SELW], BF16)
            xb = xt[:, 0:1]
            pstep = xb.ap[0][0]
            for h in range(2):
                g = bass.AP(xb.tensor, xb.offset + h * NT + j * 128,
                            [[pstep, 128], [80, 3], [1, 3], [1, 128]])
                nc.vector.tensor_mul(xts[:, h * SELW:(h + 1) * SELW], g, S[:])
            z = zp.tile([128, DIM], F32)
            for k in range(9):
                for h in range(2):
                    nc.tensor.matmul(
                        z[:],
                        xts[:, h * SELW + k * 128:h * SELW + (k + 1) * 128],
                        wt[:, (2 * k + h) * DIM:(2 * k + h + 1) * DIM],
                        start=(k == 0 and h == 0), stop=(k == 8 and h == 1))
            # per-pixel int8 quantization: step = max|z| / 127 (clamped),
            # q = round_to_nearest(z / step)
            rm = rmp.tile([128, 2], F32)
            nc.vector.tensor_reduce(rm[:, 0:1], z[:], mybir.AxisListType.X,
                                    mybir.AluOpType.max,
                                    apply_absolute_value=True)
            nc.vector.tensor_scalar(sck[:, j:j + 1], rm[:, 0:1],
                                    1.0 / 127.0, 1e-30,
                                    mybir.AluOpType.mult, mybir.AluOpType.max)
            nc.vector.reciprocal(rm[:, 1:2], sck[:, j:j + 1])
            outt = outp.tile([128, DIM], I8)
            nc.vector.tensor_scalar_mul(outt[:], z[:], rm[:, 1:2])
            nc.sync.dma_start(qout_d[j * 128:(j + 1) * 128, :], outt[:])
        nc.sync.dma_start(qout_d[NPIX:QROWS, :], sck[:].bitcast(I8))
    nc.compile()
    return nc


_NC_CACHE = None


def _get_nc():
    global _NC_CACHE
    if _NC_CACHE is None:
        _NC_CACHE = _build_nc()
    return _NC_CACHE


def _prep_core(x, seg_mask, core):
    b, r0 = core // 2, 40 * (core % 2)
    xp = np.pad(x[b], ((1, 1), (0, 0), (0, 0)))        # [82,80,256]
    strip = xp[r0:r0 + 42].reshape(42 * W, CH)
    sp = np.zeros((NT, CH), np.float32)
    sp[1:1 + 42 * W] = strip
    # per-strip-position int8 quantization of x; steps fold into sel below
    pstep = np.maximum(np.abs(sp).max(1), 1e-20) / 127.0   # [NT]
    xqT = np.rint(sp / pstep[:, None]).astype(np.int8).T   # [CH, NT]
    xt = np.ascontiguousarray(
        np.concatenate([xqT[:128], xqT[128:]], axis=1))    # [128, 2NT] int8

    pads = np.pad(seg_mask[b], ((1, 1), (1, 1), (0, 0)))  # [82,82,22]
    mc = seg_mask[b][r0:r0 + 40]                          # [40,80,22]
    smax = mc.max(-1, keepdims=True)
    eq = (mc == smax).astype(np.float32)
    sel = np.empty((40, 80, 9), np.float32)
    for k in range(9):
        di, dj = k // 3 - 1, k % 3 - 1
        sel[..., k] = (eq * pads[r0 + 1 + di:r0 + 41 + di,
                                 1 + dj:81 + dj]).sum(-1)
    cnt = (sel != 0).astype(np.float32).sum(-1, keepdims=True)
    selp = sel * (9.0 / np.maximum(cnt, 1.0))
    # fold the neighbor pixel's dequant step into the sel coefficient:
    # patch element (a,b) of center q reads strip position q + a*80 + b
    selp = selp.reshape(NPIX, 9)
    q = np.arange(NPIX)
    for k in range(9):
        selp[:, k] *= pstep[q + (k // 3) * 80 + (k % 3)]
    # [NTILE, 9, 128]: k-major, pixel-in-tile minor
    selt = np.ascontiguousarray(
        selp.reshape(NTILE, 128, 9).transpose(0, 2, 1)
    ).astype(BF16NP)
    return xt, selt


def _prep_in_maps(x, seg_mask, conv_w):
    w9 = conv_w.reshape(CH, 9, DIM)
    # [128, 9, 2, 256]: per k, both ch halves adjacent
    wt = np.ascontiguousarray(
        np.stack([w9[:128], w9[128:]], axis=2).reshape(128, 18 * DIM)
    ).astype(BF16NP)

    in_maps = []
    for core in range(8):
        xt, selt = _prep_core(x, seg_mask, core)
        inx = np.empty((128, TOTC), np.int8)
        inx[:, :XC] = xt
        inx[:, XC:XC + SC] = selt.reshape(128, SC // 2).view(np.int8)
        inx[:, XC + SC:] = wt[core * 16:(core + 1) * 16].reshape(
            128, WC // 2).view(np.int8)
        in_maps.append({"inx": inx})
    return in_maps


def kernel(x, seg_mask, conv_w):
    x = np.asarray(x, np.float32)
    seg_mask = np.asarray(seg_mask, np.float32)
    conv_w = np.asarray(conv_w, np.float32)

    in_maps = _prep_in_maps(x, seg_mask, conv_w)
    nc = _get_nc()
    res = run_bass_kernel_spmd(nc, in_maps, core_ids=list(range(8)))

    out = np.empty((B, H, W, DIM), np.float32)
    for core in range(8):
        b, r0 = core // 2, 40 * (core % 2)
        arr = res.results[core]["qout"]
        q = arr[:NPIX].astype(np.float32)
        step = np.frombuffer(arr[NPIX:].tobytes(), np.float32).reshape(
            128, NTILE).T.reshape(NPIX, 1)
        out[b, r0:r0 + 40] = (q * step).reshape(ROWS, W, DIM)
    return out


# revision 29
# speedup vs baseline: 1714.3215x; 1714.3215x over previous
import sys
from contextlib import ExitStack

import numpy as np
import ml_dtypes

sys.path.insert(0, "/opt/trn_rl_repo")

try:
    import jax
    jax.config.update("jax_compilation_cache_dir", "/tmp/jax_cc_cache")
    jax.config.update("jax_persistent_cache_min_compile_time_secs", 0.0)
    jax.config.update("jax_persistent_cache_min_entry_size_bytes", 0)
except Exception:
    pass

import concourse.bass as bass
import concourse.tile as tile
from concourse import bacc, mybir
from concourse.bass_utils import run_bass_kernel_spmd

B, H, W, CH = 4, 80, 80, 256
NCLS, DIM = 22, 256
ROWS = 40            # rows per core
NPIX = ROWS * W      # 3200 output pixels per core
NT = (ROWS + 2) * W + 2   # 3362 strip positions (1 halo row each side + 1 elem pad)
NTILE = NPIX // 128  # 25 output tiles of 128 pixels
SELW = 9 * 128       # per-tile selp row width (k-major, pixel minor)
# merged-input column layout (int8 bytes): [x strip int8 | sel bf16 | w bf16]
XC = 2 * NT                   # 6724 int8 cols of per-pixel-quantized x
SC = NTILE * SELW * 2 // 128  # 450 byte-cols of sel (bf16)
WC = 16 * 18 * DIM * 2 // 128  # 1152 byte-cols of weight shard (bf16)
TOTC = XC + SC + WC           # 8326
# packed output layout, partition-major [128, OUTW]: 25 blocks of 224 bytes
# (8 pixels' 7-bit dims packed per 7 bytes), then 25 f32 scales as bytes
PKW = NTILE * (DIM // 8) * 7  # 5600 packed bytes per partition
SCW = NTILE * 4               # 100 f32-scale bytes per partition
OUTW = PKW + SCW              # 5700
F32 = mybir.dt.float32
I8 = mybir.dt.int8
BF16 = mybir.dt.bfloat16
BF16NP = ml_dtypes.bfloat16


def _build_nc():
    nc = bacc.Bacc("TRN2", target_bir_lowering=False, debug=False,
                   enable_asserts=True, num_devices=8)
    # single merged int8 input: quantized x cols [0:XC), sel bytes
    # [XC:XC+SC), 1/8 weight-shard bytes [XC+SC:TOTC) (AllGather assembles)
    inx_d = nc.dram_tensor("inx", [128, TOTC], I8, kind="ExternalInput").ap()
    # single merged output: per-partition 7-bit-packed quantized values
    # followed by the f32 per-pixel quant steps (bitcast to bytes)
    qout_d = nc.dram_tensor("qout", [128, OUTW], I8,
                            kind="ExternalOutput").ap()

    with tile.TileContext(nc) as tc, ExitStack() as ctx:
        dramp = ctx.enter_context(tc.tile_pool(name="dramp", bufs=1,
                                               space="DRAM"))
        xp = ctx.enter_context(tc.tile_pool(name="xp", bufs=1))
        wp = ctx.enter_context(tc.tile_pool(name="wp", bufs=1))
        stp = ctx.enter_context(tc.tile_pool(name="stp", bufs=1))
        Sp = ctx.enter_context(tc.tile_pool(name="Sp", bufs=3))
        xtsp = ctx.enter_context(tc.tile_pool(name="xtsp", bufs=3))
        rmp = ctx.enter_context(tc.tile_pool(name="rmp", bufs=3))
        sckp = ctx.enter_context(tc.tile_pool(name="sckp", bufs=1))
        up = ctx.enter_context(tc.tile_pool(name="up", bufs=1))
        pkp = ctx.enter_context(tc.tile_pool(name="pkp", bufs=1))
        tmpp = ctx.enter_context(tc.tile_pool(name="tmpp", bufs=2))
        zp = ctx.enter_context(tc.tile_pool(name="zp", bufs=6, space="PSUM"))

        xt = xp.tile([128, XC], I8)
        wt = wp.tile([128, 18 * DIM], BF16)
        selt = stp.tile([1, NTILE * SELW], BF16)

        win = dramp.tile([16, 18 * DIM], BF16)
        wfull = dramp.tile([128, 18 * DIM], BF16)
        nc.gpsimd.dma_start(win[:], inx_d[:, XC + SC:TOTC].bitcast(BF16))
        nc.gpsimd.collective_compute(
            "AllGather", mybir.AluOpType.bypass,
            replica_groups=[list(range(8))],
            ins=[win.opt()], outs=[wfull.opt()])

        nc.sync.dma_start(selt[:], inx_d[:, XC:XC + SC].bitcast(BF16))
        # x chunk 0 first (tile 0's multiply needs it), then weights, then rest
        bnds = [0, 850, 1700, 2550, NT]
        for h in range(2):
            nc.sync.dma_start(xt[:, h * NT:h * NT + bnds[1]],
                              inx_d[:, h * NT:h * NT + bnds[1]])
        for k in range(9):
            nc.sync.dma_start(wt[:, k * 512:(k + 1) * 512],
                              wfull[:, k * 512:(k + 1) * 512])
        for ci in range(1, 4):
            for h in range(2):
                a, b = h * NT + bnds[ci], h * NT + bnds[ci + 1]
                nc.sync.dma_start(xt[:, a:b], inx_d[:, a:b])

        sck = sckp.tile([128, NTILE], F32)
        uall = up.tile([128, NTILE * DIM], I8)
        for j in range(NTILE):
            S = Sp.tile([128, SELW], BF16)
            nc.gpsimd.partition_broadcast(
                S[:], selt[0:1, j * SELW:(j + 1) * SELW])
            xts = xtsp.tile([128, 2 * SELW], BF16)
            xb = xt[:, 0:1]
            pstep = xb.ap[0][0]
            for h in range(2):
                g = bass.AP(xb.tensor, xb.offset + h * NT + j * 128,
                            [[pstep, 128], [80, 3], [1, 3], [1, 128]])
                nc.vector.tensor_mul(xts[:, h * SELW:(h + 1) * SELW], g, S[:])
            z = zp.tile([128, DIM], F32)
            for k in range(9):
                for h in range(2):
                    nc.tensor.matmul(
                        z[:],
                        xts[:, h * SELW + k * 128:h * SELW + (k + 1) * 128],
                        wt[:, (2 * k + h) * DIM:(2 * k + h + 1) * DIM],
                        start=(k == 0 and h == 0), stop=(k == 8 and h == 1))
            # per-pixel 7-bit quantization: step = max|z| / 63 (clamped),
            # u = round_to_nearest(z / step) + 64 in [0, 127]
            rm = rmp.tile([128, 2], F32)
            nc.vector.tensor_reduce(rm[:, 0:1], z[:], mybir.AxisListType.X,
                                    mybir.AluOpType.max,
                                    apply_absolute_value=True)
            nc.vector.tensor_scalar(sck[:, j:j + 1], rm[:, 0:1],
                                    1.0 / 63.0, 1e-30,
                                    mybir.AluOpType.mult, mybir.AluOpType.max)
            nc.vector.reciprocal(rm[:, 1:2], sck[:, j:j + 1])
            nc.vector.tensor_scalar(uall[:, j * DIM:(j + 1) * DIM], z[:],
                                    rm[:, 1:2], 64.0,
                                    mybir.AluOpType.mult, mybir.AluOpType.add)
        # pack 8x7-bit into 7 bytes: b_i = (u_i << (i+1)) | (u_{i+1} >> (6-i))
        pk = pkp.tile([128, PKW], I8)
        ub = uall[:, 0:1]
        upstep = ub.ap[0][0]
        pb = pk[:, 0:1]
        ppstep = pb.ap[0][0]
        NG = DIM // 8  # 32 groups per block

        def uview(i):
            return bass.AP(ub.tensor, ub.offset + i,
                           [[upstep, 128], [DIM, NTILE], [8, NG]])

        def bview(i):
            return bass.AP(pb.tensor, pb.offset + i,
                           [[ppstep, 128], [7 * NG, NTILE], [7, NG]])

        for i in range(7):
            t1 = tmpp.tile([128, NTILE * NG], I8)
            t2 = tmpp.tile([128, NTILE * NG], I8)
            tb = t1[:, 0:1]
            tp = tb.ap[0][0]
            t1v = bass.AP(tb.tensor, tb.offset,
                          [[tp, 128], [NG, NTILE], [1, NG]])
            tb2 = t2[:, 0:1]
            t2v = bass.AP(tb2.tensor, tb2.offset,
                          [[tb2.ap[0][0], 128], [NG, NTILE], [1, NG]])
            nc.vector.tensor_scalar(t1v, uview(i + 1), 6 - i, None,
                                    mybir.AluOpType.logical_shift_right)
            nc.vector.tensor_scalar(t2v, uview(i), i + 1, None,
                                    mybir.AluOpType.logical_shift_left)
            nc.vector.tensor_tensor(bview(i), t2v, t1v,
                                    mybir.AluOpType.bitwise_or)
        nc.sync.dma_start(qout_d[:, 0:PKW], pk[:])
        nc.sync.dma_start(qout_d[:, PKW:OUTW], sck[:].bitcast(I8))
    nc.compile()
    return nc


_NC_CACHE = None


def _get_nc():
    global _NC_CACHE
    if _NC_CACHE is None:
        _NC_CACHE = _build_nc()
    return _NC_CACHE


def _prep_core(x, seg_mask, core):
    b, r0 = core // 2, 40 * (core % 2)
    xp = np.pad(x[b], ((1, 1), (0, 0), (0, 0)))        # [82,80,256]
    strip = xp[r0:r0 + 42].reshape(42 * W, CH)
    sp = np.zeros((NT, CH), np.float32)
    sp[1:1 + 42 * W] = strip
    # per-strip-position int8 quantization of x; steps fold into sel below
    pstep = np.maximum(np.abs(sp).max(1), 1e-20) / 127.0   # [NT]
    xqT = np.rint(sp / pstep[:, None]).astype(np.int8).T   # [CH, NT]
    xt = np.ascontiguousarray(
        np.concatenate([xqT[:128], xqT[128:]], axis=1))    # [128, 2NT] int8

    pads = np.pad(seg_mask[b], ((1, 1), (1, 1), (0, 0)))  # [82,82,22]
    mc = seg_mask[b][r0:r0 + 40]                          # [40,80,22]
    smax = mc.max(-1, keepdims=True)
    eq = (mc == smax).astype(np.float32)
    sel = np.empty((40, 80, 9), np.float32)
    for k in range(9):
        di, dj = k // 3 - 1, k % 3 - 1
        sel[..., k] = (eq * pads[r0 + 1 + di:r0 + 41 + di,
                                 1 + dj:81 + dj]).sum(-1)
    cnt = (sel != 0).astype(np.float32).sum(-1, keepdims=True)
    selp = sel * (9.0 / np.maximum(cnt, 1.0))
    # fold the neighbor pixel's dequant step into the sel coefficient:
    # patch element (a,b) of center q reads strip position q + a*80 + b
    selp = selp.reshape(NPIX, 9)
    q = np.arange(NPIX)
    for k in range(9):
        selp[:, k] *= pstep[q + (k // 3) * 80 + (k % 3)]
    # [NTILE, 9, 128]: k-major, pixel-in-tile minor
    selt = np.ascontiguousarray(
        selp.reshape(NTILE, 128, 9).transpose(0, 2, 1)
    ).astype(BF16NP)
    return xt, selt


def _prep_in_maps(x, seg_mask, conv_w):
    w9 = conv_w.reshape(CH, 9, DIM)
    # [128, 9, 2, 256]: per k, both ch halves adjacent
    wt = np.ascontiguousarray(
        np.stack([w9[:128], w9[128:]], axis=2).reshape(128, 18 * DIM)
    ).astype(BF16NP)

    in_maps = []
    for core in range(8):
        xt, selt = _prep_core(x, seg_mask, core)
        inx = np.empty((128, TOTC), np.int8)
        inx[:, :XC] = xt
        inx[:, XC:XC + SC] = selt.reshape(128, SC // 2).view(np.int8)
        inx[:, XC + SC:] = wt[core * 16:(core + 1) * 16].reshape(
            128, WC // 2).view(np.int8)
        in_maps.append({"inx": inx})
    return in_maps


def kernel(x, seg_mask, conv_w):
    x = np.asarray(x, np.float32)
    seg_mask = np.asarray(seg_mask, np.float32)
    conv_w = np.asarray(conv_w, np.float32)

    in_maps = _prep_in_maps(x, seg_mask, conv_w)
    nc = _get_nc()
    res = run_bass_kernel_spmd(nc, in_maps, core_ids=list(range(8)))

    out = np.empty((B, H, W, DIM), np.float32)
    for core in range(8):
        b, r0 = core // 2, 40 * (core % 2)
        arr = res.results[core]["qout"].view(np.uint8)
        pk = arr[:, :PKW].reshape(128, NTILE, DIM // 8, 7).astype(np.uint16)
        bb = [pk[..., i] for i in range(7)]
        u = np.empty((128, NTILE, DIM // 8, 8), np.int32)
        u[..., 0] = bb[0] >> 1
        for i in range(1, 7):
            u[..., i] = (((bb[i - 1] & ((1 << i) - 1)) << (7 - i))
                         | (bb[i] >> (i + 1)))
        u[..., 7] = bb[6] & 127
        q = (u - 64).reshape(128, NTILE, DIM).transpose(1, 0, 2).reshape(
            NPIX, DIM).astype(np.float32)
        step = np.ascontiguousarray(arr[:, PKW:OUTW]).view(
            np.float32).T.reshape(NPIX, 1)
        out[b, r0:r0 + 40] = (q * step).reshape(ROWS, W, DIM)
    return out


# revision 30
# speedup vs baseline: 4078.4442x; 2.3790x over previous
import sys
from contextlib import ExitStack

import numpy as np
import ml_dtypes

sys.path.insert(0, "/opt/trn_rl_repo")

try:
    import jax
    jax.config.update("jax_compilation_cache_dir", "/tmp/jax_cc_cache")
    jax.config.update("jax_persistent_cache_min_compile_time_secs", 0.0)
    jax.config.update("jax_persistent_cache_min_entry_size_bytes", 0)
except Exception:
    pass

import concourse.bass as bass
import concourse.tile as tile
from concourse import bacc, mybir
from concourse.bass_utils import run_bass_kernel_spmd

B, H, W, CH = 4, 80, 80, 256
NCLS, DIM = 22, 256
ROWS = 40            # rows per core
NPIX = ROWS * W      # 3200 output pixels per core
NT = (ROWS + 2) * W + 2   # 3362 strip positions (1 halo row each side + 1 elem pad)
NTILE = NPIX // 128  # 25 output tiles of 128 pixels
SELW = 9 * 128       # per-tile selp row width (k-major, pixel minor)
# merged-input column layout (bf16): [x strip | sel | full weights]
XC = 2 * NT               # 6724 cols of x data
SC = NTILE * SELW // 128  # 225 cols of sel data
WC = 18 * DIM             # 4608 cols of full weights
TOTC = XC + SC + WC       # 11557
F32 = mybir.dt.float32
F16 = mybir.dt.float16
BF16 = mybir.dt.bfloat16
BF16NP = ml_dtypes.bfloat16


def _build_nc():
    nc = bacc.Bacc("TRN2", target_bir_lowering=False, debug=False,
                   enable_asserts=True, num_devices=8)
    # single merged bf16 input: x strip cols [0:XC), sel cols [XC:XC+SC),
    # full replicated weights cols [XC+SC:TOTC)
    inx_d = nc.dram_tensor("inx", [128, TOTC], BF16, kind="ExternalInput").ap()
    out_d = nc.dram_tensor("out", [NPIX, DIM], F16, kind="ExternalOutput").ap()

    with tile.TileContext(nc) as tc, ExitStack() as ctx:
        xp = ctx.enter_context(tc.tile_pool(name="xp", bufs=1))
        wp = ctx.enter_context(tc.tile_pool(name="wp", bufs=1))
        stp = ctx.enter_context(tc.tile_pool(name="stp", bufs=1))
        Sp = ctx.enter_context(tc.tile_pool(name="Sp", bufs=3))
        xtsp = ctx.enter_context(tc.tile_pool(name="xtsp", bufs=3))
        outp = ctx.enter_context(tc.tile_pool(name="outp", bufs=3))
        zp = ctx.enter_context(tc.tile_pool(name="zp", bufs=6, space="PSUM"))

        xt = xp.tile([128, XC], BF16)
        wt = wp.tile([128, WC], BF16)
        selt = stp.tile([1, NTILE * SELW], BF16)

        nc.sync.dma_start(selt[:], inx_d[:, XC:XC + SC])
        # x chunk 0 first (tile 0's multiply needs it), then weights, then rest
        bnds = [0, 850, 1700, 2550, NT]
        for h in range(2):
            nc.sync.dma_start(xt[:, h * NT:h * NT + bnds[1]],
                              inx_d[:, h * NT:h * NT + bnds[1]])
        for k in range(9):
            nc.sync.dma_start(wt[:, k * 512:(k + 1) * 512],
                              inx_d[:, XC + SC + k * 512:XC + SC + (k + 1) * 512])
        for ci in range(1, 4):
            for h in range(2):
                a, b = h * NT + bnds[ci], h * NT + bnds[ci + 1]
                nc.sync.dma_start(xt[:, a:b], inx_d[:, a:b])

        for j in range(NTILE):
            S = Sp.tile([128, SELW], BF16)
            nc.gpsimd.partition_broadcast(
                S[:], selt[0:1, j * SELW:(j + 1) * SELW])
            xts = xtsp.tile([128, 2 * SELW], BF16)
            xb = xt[:, 0:1]
            pstep = xb.ap[0][0]
            for h in range(2):
                g = bass.AP(xb.tensor, xb.offset + h * NT + j * 128,
                            [[pstep, 128], [80, 3], [1, 3], [1, 128]])
                nc.vector.tensor_mul(xts[:, h * SELW:(h + 1) * SELW], g, S[:])
            z = zp.tile([128, DIM], F32)
            for k in range(9):
                for h in range(2):
                    nc.tensor.matmul(
                        z[:],
                        xts[:, h * SELW + k * 128:h * SELW + (k + 1) * 128],
                        wt[:, (2 * k + h) * DIM:(2 * k + h + 1) * DIM],
                        start=(k == 0 and h == 0), stop=(k == 8 and h == 1))
            outt = outp.tile([128, DIM], F16)
            nc.scalar.copy(outt[:], z[:])
            nc.sync.dma_start(out_d[j * 128:(j + 1) * 128, :], outt[:])
    nc.compile()
    return nc


_NC_CACHE = None


def _get_nc():
    global _NC_CACHE
    if _NC_CACHE is None:
        _NC_CACHE = _build_nc()
    return _NC_CACHE


def _prep_core(x, seg_mask, core):
    b, r0 = core // 2, 40 * (core % 2)
    xp = np.pad(x[b], ((1, 1), (0, 0), (0, 0)))        # [82,80,256]
    strip = xp[r0:r0 + 42].reshape(42 * W, CH)
    sp = np.zeros((NT, CH), np.float32)
    sp[1:1 + 42 * W] = strip
    spT = sp.T
    xt = np.ascontiguousarray(
        np.concatenate([spT[:128], spT[128:]], axis=1)).astype(BF16NP)

    pads = np.pad(seg_mask[b], ((1, 1), (1, 1), (0, 0)))  # [82,82,22]
    mc = seg_mask[b][r0:r0 + 40]                          # [40,80,22]
    smax = mc.max(-1, keepdims=True)
    eq = (mc == smax).astype(np.float32)
    sel = np.empty((40, 80, 9), np.float32)
    for k in range(9):
        di, dj = k // 3 - 1, k % 3 - 1
        sel[..., k] = (eq * pads[r0 + 1 + di:r0 + 41 + di,
                                 1 + dj:81 + dj]).sum(-1)
    cnt = (sel != 0).astype(np.float32).sum(-1, keepdims=True)
    selp = sel * (9.0 / np.maximum(cnt, 1.0))
    # [NTILE, 9, 128]: k-major, pixel-in-tile minor
    selt = np.ascontiguousarray(
        selp.reshape(NTILE, 128, 9).transpose(0, 2, 1)
    ).astype(BF16NP).reshape(128, SC)
    return xt, selt


def _prep_in_maps(x, seg_mask, conv_w):
    w9 = conv_w.reshape(CH, 9, DIM)
    # [128, 9, 2, 256]: per k, both ch halves adjacent
    wt = np.ascontiguousarray(
        np.stack([w9[:128], w9[128:]], axis=2).reshape(128, WC)
    ).astype(BF16NP)

    in_maps = []
    for core in range(8):
        xt, selt = _prep_core(x, seg_mask, core)
        inx = np.empty((128, TOTC), BF16NP)
        inx[:, :XC] = xt
        inx[:, XC:XC + SC] = selt
        inx[:, XC + SC:] = wt
        in_maps.append({"inx": inx})
    return in_maps


def kernel(x, seg_mask, conv_w):
    x = np.asarray(x, np.float32)
    seg_mask = np.asarray(seg_mask, np.float32)
    conv_w = np.asarray(conv_w, np.float32)

    in_maps = _prep_in_maps(x, seg_mask, conv_w)
    nc = _get_nc()
    res = run_bass_kernel_spmd(nc, in_maps, core_ids=list(range(8)))

    out = np.empty((B, H, W, DIM), np.float32)
    for core in range(8):
        b, r0 = core // 2, 40 * (core % 2)
        out[b, r0:r0 + 40] = res.results[core]["out"].astype(
            np.float32).reshape(ROWS, W, DIM)
    return out
